# revision 14
# baseline (speedup 1.0000x reference)
"""Trainium2 Bass kernel: ResNet BasicBlock (conv3x3-BN-ReLU-mask-conv3x3-mask-BN-residual-ReLU).

Problem shape: x[4096, 64, 7, 7], both convs 64->64 3x3 pad 1.

Strategy (pure data parallel, 8 cores, 512 images/core):
  * Channels live on SBUF partitions. Two 64-channel image streams are
    stacked into the 128 partitions ("half0" -> partitions 0-63,
    "half1" -> 64-127) so elementwise engines run at full width.
  * A 3x3 conv is 9 shifted 64x64 matmuls accumulated in PSUM. Images are
    zero-padded to 9x9 on-chip; each tap reads a strided window of the
    padded tile. Matmul operands are bf16; accumulation stays fp32 in
    PSUM.
  * The 128x128 PE array is split into 4 64x64 quadrants via the matmul
    base partitions. Four independent tap-chains (2 pairs x 2 halves) run
    concurrently, so the array is fully utilized despite C=64.
  * Quads are processed in PAIRS with the tap loop outer across the pair:
    the second quad of each tap reuses the stationary weights already in
    the PE array, and its redundant LDWEIGHTS are deleted from the BIR
    post-schedule (the PE retains loaded weights - verified on HW).
  * The software pipeline is the classic pending-quad scheme: conv2 of
    pair k-1 is interleaved after conv1 of pair k, so PSUM banks and
    engines stay busy with no phase barriers.
  * BN scales are folded into the conv weights on the host; BN shifts are
    per-partition bias/scalar operands. The identity residual is added in
    exact fp32 by VectorE directly into PSUM before the final relu.
  * The output travels back as bf16 (<=2^-9 relative rounding - well
    inside the error budget), halving outbound HBM traffic; the last
    quad's output DMAs split across the sync and gpsimd rings to shorten
    the kernel tail.
  * Prologue: w1 rides first on the ACT HWDGE ring, a strided-destination
    CAST warmup absorbs the ~3-8us one-time cost the first real pad copy
    would otherwise pay, and ~60 dummy matmuls bridge the DMA wait so the
    PE HAM clock gate is already at 2.4 GHz when the real stream begins.
  * The critic masks only touch batch element 0: every core runs the same
    mask multiply on its first image, but cores 1-7 get all-ones masks.
"""

import ml_dtypes
import numpy as np

import concourse.bass as bass  # noqa: F401  (engine namespaces live on the nc object)
import concourse.tile as tile
from concourse import bacc, mybir
from concourse.bass_utils import run_bass_kernel_spmd

F32 = mybir.dt.float32
BF16 = mybir.dt.bfloat16
NP_BF16 = ml_dtypes.bfloat16
EPS = 1e-5
B, C, H, W = 4096, 64, 7, 7
NCORES = 8
BPC = B // NCORES          # 512 images per core
SLOTS = BPC // 2           # 256 image slots per half-stream
NBUF = 4                   # padded-tile ring depth (pair-deep pipeline)
NWARM = 60                 # HAM warmup matmuls
# Quads: (base_slot, n_images_per_chain). n=10 -> 490 of 512 psum words.
QUADS = [(20 * v, 10) for v in range(12)] + [(240, 8)]
NMAX = 10
# quad pairs for the tap-outer weight reuse (the ragged 13th quad stands
# alone)
PAIRS = [(0, 1), (2, 3), (4, 5), (6, 7), (8, 9), (10, 11), (12,)]

# (pair_in_quad, half, colgroup): the 4 concurrent chains of a quad.
CHAINS = [(0, 0, 0), (1, 1, 0), (0, 1, 1), (1, 0, 1)]

_CACHE = {}


def _psum_view(ps, j, n):
    """[128, n, 7, 7] view of pair j's bank of a [128, 2, 512] psum tile."""
    return ps[:, j, 0:n * H * W].rearrange(
        "p (i h w) -> p i h w", i=n, h=H, w=W)


def _dedup_ldweights(nc):
    """Remove InstLdweights whose matmul reuses the weights already loaded
    into the same PE quadrant (tile_position). The PE array retains its
    stationary operand between matmuls, so a matmul with no preceding
    LDWEIGHTS streams against the previously loaded weights (verified on
    hardware). Sync waits/updates on a removed LDWEIGHTS migrate to its
    matmul."""
    kept = removed = 0
    for f in nc.m.functions:
        for b in f.blocks:
            insts = list(b.instructions)
            last = {}
            dead = []
            i = 0
            while i < len(insts):
                ins = insts[i]
                if isinstance(ins, mybir.InstLdweights):
                    assert i + 1 < len(insts), "trailing LDWEIGHTS"
                    mm = insts[i + 1]
                    assert isinstance(mm, mybir.InstMatmult), (
                        f"LDWEIGHTS not followed by matmul: {type(mm).__name__}")
                    sig = str(mm.ins[1])
                    tp = tuple(mm.tile_position)
                    if last.get(tp) == sig:
                        si = ins.sync_info
                        if si is not None and (len(si.on_wait) or len(si.on_update)):
                            msi = mm.sync_info
                            ow = list(si.on_wait)
                            ou = list(si.on_update)
                            if msi is not None:
                                ow += list(msi.on_wait)
                                ou += list(msi.on_update)
                            mm.sync_info = mybir.SyncInfo(on_wait=ow, on_update=ou)
                        dead.append(ins)
                        removed += 1
                    else:
                        last[tp] = sig
                        kept += 1
                    i += 2
                    continue
                assert not isinstance(ins, mybir.InstMatmult), "matmul without LDWEIGHTS"
                i += 1
            for d in dead:
                b.instructions.remove(d)
    assert kept + removed == 936 + NWARM and removed >= 350, (kept, removed)


def _build():
    nc = bacc.Bacc("TRN2", target_bir_lowering=False, debug=False,
                   num_devices=NCORES)
    x_d = nc.dram_tensor("x", [128, SLOTS, H, W], F32, kind="ExternalInput")
    w1_d = nc.dram_tensor("w1", [128, 9, 64], BF16, kind="ExternalInput")
    w2_d = nc.dram_tensor("w2", [128, 9, 64], BF16, kind="ExternalInput")
    cst_d = nc.dram_tensor("cst", [128, 2], F32, kind="ExternalInput")
    msk_d = nc.dram_tensor("msk", [64, 2, H, W], F32, kind="ExternalInput")
    o_d = nc.dram_tensor("o", [128, SLOTS, H, W], BF16, kind="ExternalOutput")

    with tile.TileContext(nc) as tc:
        with (
            tc.tile_pool(name="singles", bufs=1) as singles,
            tc.tile_pool(name="xin", bufs=8) as xin_pool,
            tc.tile_pool(name="outp", bufs=3) as out_pool,
            tc.tile_pool(name="pads", bufs=1) as pad_pool,
            tc.tile_pool(name="ps1", bufs=2, space="PSUM") as ps1_pool,
            tc.tile_pool(name="ps2", bufs=2, space="PSUM") as ps2_pool,
        ):
            w1_sb = singles.tile([128, 9, 64], BF16, name="w1_sb")
            w2_sb = singles.tile([128, 9, 64], BF16, name="w2_sb")
            cst_sb = singles.tile([128, 2], F32, name="cst_sb")
            msk_sb = singles.tile([64, 2, H, W], F32, name="msk_sb")
            warm_sb = singles.tile([128, 1], F32, name="warm_sb")
            wrm_src = singles.tile([128, 8 * 9 * 9], F32, name="wrm_src")
            wrm_bf = singles.tile([128, 64], BF16, name="wrm_bf")

            xpads, y1pads = [], []
            for i in range(NBUF):
                xp = pad_pool.tile([128, 2 * NMAX, 9, 9], BF16,
                                   name=f"xpad{i}", tag=f"xpad{i}")
                yp = pad_pool.tile([128, 2 * NMAX, 9, 9], BF16,
                                   name=f"y1pad{i}", tag=f"y1pad{i}")
                xpads.append(xp)
                y1pads.append(yp)

            def emit_conv_tap(src_pad, w_sb, ps, t, n, start, stop):
                dh, dw = t // 3, t % 3
                for (j, half, cg) in CHAINS:
                    rhs = src_pad[64 * half:64 * half + 64,
                                  n * j:n * (j + 1), dh:dh + 7, dw:dw + 7]
                    lhsT = w_sb[64 * half:64 * half + 64, t, :]
                    out = ps[64 * cg:64 * cg + 64, j, 0:n * H * W]
                    nc.tensor.matmul(out, lhsT, rhs, start=start, stop=stop)

            def emit_drain(v, ps1):
                base, n = QUADS[v]
                yp = y1pads[v % NBUF]
                for j in range(2):
                    nc.scalar.activation(
                        out=yp[:, n * j:n * (j + 1), 1:8, 1:8],
                        in_=_psum_view(ps1, j, n),
                        func=mybir.ActivationFunctionType.Relu,
                        bias=cst_sb[:, 0:1], scale=1.0)
                if v == 0:
                    # critic mask 1 on relu(bn1(conv1)) of batch elem 0
                    tgt = yp[0:64, 0, 1:8, 1:8]
                    nc.vector.tensor_mul(tgt, tgt, msk_sb[:, 0, :, :])

            def emit_finish(v, ps2, xin_q):
                base, n = QUADS[v]
                if v == 0:
                    # critic mask 2 on conv2 output of batch element 0
                    tgt = ps2[0:64, 0, 0:H * W].rearrange(
                        "p (h w) -> p h w", h=H, w=W)
                    nc.vector.tensor_mul(tgt, tgt, msk_sb[:, 1, :, :])
                views = [_psum_view(ps2, j, n) for j in range(2)]
                out_q = out_pool.tile([128, 2 * NMAX, H, W], BF16,
                                      name="out_q")
                for j in range(2):
                    nc.vector.tensor_add(
                        views[j], views[j], xin_q[:, n * j:n * (j + 1)])
                nc.vector.tensor_scalar(
                    out_q[:, 0:n], views[0],
                    cst_sb[:, 1:2], 0.0,
                    mybir.AluOpType.add, mybir.AluOpType.max)
                nc.scalar.activation(
                    out=out_q[:, n:2 * n], in_=views[1],
                    func=mybir.ActivationFunctionType.Relu,
                    bias=cst_sb[:, 1:2], scale=1.0)
                if v == len(QUADS) - 1:
                    # final quad: split across two rings so the tail's
                    # serial descriptor issues overlap
                    nc.sync.dma_start(o_d[:, base:base + n], out_q[:, 0:n])
                    nc.gpsimd.dma_start(o_d[:, base + n:base + 2 * n],
                                        out_q[:, n:2 * n])
                else:
                    nc.sync.dma_start(o_d[:, base:base + 2 * n],
                                      out_q[:, 0:2 * n])

            xin_qs = {}

            def emit_in_dma(v, q):
                base, n = QUADS[v]
                t = xin_pool.tile([128, 2 * NMAX, H, W], F32, name="xin_q")
                q.dma_start(t[:, 0:2 * n], x_d[:, base:base + 2 * n])
                xin_qs[v] = t

            # ---------------- prologue ----------------
            # DVE: warmup data, then the first pad border zero, then a
            # strided-dst CAST warmup (xpad2's interior is garbage until
            # its real pad copy; its border memset comes later)
            nc.vector.memset(wrm_src[:], 0.0)
            nc.vector.tensor_copy(wrm_bf[:], wrm_src[:, 0:64])
            nc.vector.memset(xpads[0][:], 0.0)
            nc.vector.tensor_copy(
                xpads[2][:, 0:8, 1:8, 1:8],
                wrm_src[:, 0:392].rearrange("p (i h w) -> p i h w",
                                            i=8, h=7, w=7))
            # ACT ring: w1 first (gates the first LDWEIGHTS), then the
            # ACT-table prewarm
            nc.scalar.dma_start(w1_sb[:], w1_d[:])
            nc.scalar.memzero(warm_sb[:])
            # sync ring: first quads' x
            emit_in_dma(0, nc.sync)
            emit_in_dma(1, nc.sync)
            # constants on the SWDGE ring
            nc.gpsimd.dma_start(cst_sb[:], cst_d[:])
            nc.gpsimd.dma_start(w2_sb[:], w2_d[:])
            nc.gpsimd.dma_start(msk_sb[:], msk_d[:])
            emit_in_dma(2, nc.scalar)
            emit_in_dma(3, nc.scalar)
            # HAM warmup: dummy matmuls bridge the DMA wait so the real
            # stream starts at 2.4 GHz
            ps_w = ps1_pool.tile([128, 2, 512], F32, name="ps1t")
            for i in range(NWARM):
                nc.tensor.matmul(ps_w[0:64, i % 2, 0:64],
                                 wrm_bf[0:64, :], wrm_bf[0:64, :],
                                 start=True, stop=True,
                                 skip_group_check=True)
            # y1pad borders off the critical path
            for yp in y1pads:
                nc.gpsimd.memset(yp[:], 0.0)

            # ---------------- main loop ----------------
            # pending conv2 work: list of (v, ps2, xin_q)
            pending = []
            dma_next = 4

            def emit_conv2_block(vs):
                for v in vs:
                    base, n = QUADS[v]
                    ps2 = ps2_pool.tile([128, 2, 512], F32, name="ps2t")
                    pending.append((v, ps2, xin_qs[v]))
                for t in range(9):
                    for (v, ps2, _) in pending:
                        emit_conv_tap(y1pads[v % NBUF], w2_sb, ps2, t,
                                      QUADS[v][1], t == 0, t == 8)
                for (v, ps2, xin_q) in pending:
                    emit_finish(v, ps2, xin_q)
                pending.clear()

            prev_pair = None
            for pair in PAIRS:
                # input DMAs two pairs ahead + pad copies for this pair
                for v in pair:
                    if dma_next < len(QUADS):
                        emit_in_dma(dma_next, nc.sync)
                        dma_next += 1
                for v in pair:
                    xp = xpads[v % NBUF]
                    if 1 <= v < NBUF:
                        nc.vector.memset(xp[:], 0.0)
                    n = QUADS[v][1]
                    nc.vector.tensor_copy(xp[:, 0:2 * n, 1:8, 1:8],
                                          xin_qs[v][:, 0:2 * n])
                # conv1, tap-outer across the pair
                ps1s = {v: ps1_pool.tile([128, 2, 512], F32, name="ps1t")
                        for v in pair}
                for t in range(9):
                    for v in pair:
                        emit_conv_tap(xpads[v % NBUF], w1_sb, ps1s[v], t,
                                      QUADS[v][1], t == 0, t == 8)
                for v in pair:
                    emit_drain(v, ps1s[v])
                # conv2 of the previous pair
                if prev_pair is not None:
                    emit_conv2_block(prev_pair)
                prev_pair = pair
            emit_conv2_block(prev_pair)

    _dedup_ldweights(nc)
    nc.compile()
    return nc


def _get_nc():
    if "nc" not in _CACHE:
        _CACHE["nc"] = _build()
    return _CACHE["nc"]


def _host_pack(x, w1, g1, b1, m1, v1, w2, g2, b2, m2, v2, mask1, mask2):
    x = np.ascontiguousarray(np.asarray(x, np.float32))
    scale1 = np.asarray(g1, np.float32) / np.sqrt(np.asarray(v1, np.float32) + EPS)
    shift1 = np.asarray(b1, np.float32) - np.asarray(m1, np.float32) * scale1
    scale2 = np.asarray(g2, np.float32) / np.sqrt(np.asarray(v2, np.float32) + EPS)
    shift2 = np.asarray(b2, np.float32) - np.asarray(m2, np.float32) * scale2

    def pack_w(w, scale):
        ws = np.asarray(w, np.float32) * scale[:, None, None, None]
        # [co, ci, kh, kw] -> [ci, tap, co], duplicated into both halves
        lhsT = ws.transpose(1, 2, 3, 0).reshape(64, 9, 64)
        return np.ascontiguousarray(np.tile(lhsT, (2, 1, 1)).astype(NP_BF16))

    wdev1, wdev2 = pack_w(w1, scale1), pack_w(w2, scale2)
    cst = np.tile(np.stack([shift1, shift2], 1), (2, 1))
    cst = np.ascontiguousarray(cst.astype(np.float32))

    # [core, pair, half, img, ch, h, w] -> [core, half*ch, slot, h, w]
    def pack_part(xp, npair, n):
        xr = xp.reshape(NCORES, npair, 2, n, C, H, W)
        return xr.transpose(0, 2, 4, 1, 3, 5, 6).reshape(
            NCORES, 128, npair * n, H, W)

    x_cores = x.reshape(NCORES, BPC, C, H, W)
    xdev = np.ascontiguousarray(np.concatenate(
        [pack_part(x_cores[:, 0:480], 24, 10),
         pack_part(x_cores[:, 480:512], 2, 8)], axis=2))

    msk0 = np.ascontiguousarray(
        np.stack([np.asarray(mask1, np.float32),
                  np.asarray(mask2, np.float32)], 1))
    msk1s = np.ones_like(msk0)

    in_maps = []
    for c in range(NCORES):
        in_maps.append({
            "x": xdev[c],
            "w1": wdev1,
            "w2": wdev2,
            "cst": cst,
            "msk": msk0 if c == 0 else msk1s,
        })
    return in_maps


def _host_unpack(results):
    o = np.stack([np.asarray(results[c]["o"], np.float32)
                  for c in range(NCORES)])

    def unpack_part(op, npair, n):
        op = op.reshape(NCORES, 2, C, npair, n, H, W)
        return op.transpose(0, 3, 1, 4, 2, 5, 6).reshape(
            NCORES, npair * 2 * n, C, H, W)

    out = np.concatenate([unpack_part(o[:, :, 0:240], 24, 10),
                          unpack_part(o[:, :, 240:256], 2, 8)], axis=1)
    return np.ascontiguousarray(out.reshape(B, C, H, W))


def run(trace=False, **inputs):
    nc = _get_nc()
    in_maps = _host_pack(**inputs)
    res = run_bass_kernel_spmd(nc, in_maps, core_ids=list(range(NCORES)),
                               trace=trace)
    return _host_unpack(res.results), res


def kernel(**inputs) -> np.ndarray:
    out, _ = run(trace=False, **inputs)
    return out


# revision 15
# speedup vs baseline: 1.3672x; 1.3672x over previous
"""Trainium2 Bass kernel: ResNet BasicBlock (conv3x3-BN-ReLU-mask-conv3x3-mask-BN-residual-ReLU).

Problem shape: x[4096, 64, 7, 7], both convs 64->64 3x3 pad 1.

Strategy (pure data parallel, 8 cores, 512 images/core):
  * Channels live on SBUF partitions. Two 64-channel image streams are
    stacked into the 128 partitions ("half0" -> partitions 0-63,
    "half1" -> 64-127) so elementwise engines run at full width.
  * A 3x3 conv is 9 shifted 64x64 matmuls accumulated in PSUM. Images are
    zero-padded to 9x9 on-chip; each tap reads a strided window of the
    padded tile. Matmul operands are bf16 (fp32 matmuls lower to two PE
    passes - 4x the cost); accumulation stays fp32 in PSUM.
  * The 128x128 PE array is split into 4 64x64 quadrants via the matmul
    base partitions (rhs base -> row group, psum base -> column group).
    Four independent tap-chains (2 pairs x 2 halves) run concurrently, so
    the array is fully utilized despite C=64.
  * BN scales are folded into the conv weights on the host; BN shifts are
    per-partition bias/scalar operands of ScalarE activations. The
    identity residual is added in exact fp32 by VectorE directly into
    PSUM before the final relu.
  * The output travels back as bf16 (<=2^-9 relative rounding - well
    inside the error budget), halving outbound HBM traffic; the last
    quad's output DMAs split across the sync and gpsimd rings so the
    kernel tail's serial descriptor issues overlap.
  * Prologue: w1 rides first on the ACT HWDGE ring (it gates the first
    LDWEIGHTS), a strided-destination CAST warmup absorbs the multi-us
    one-time cost the first real pad copy would otherwise pay, and ~60
    dummy matmuls bridge the input-DMA wait so the PE HAM clock gate is
    already at 2.4 GHz when the real matmul stream begins.
  * The critic masks only touch batch element 0: every core runs the same
    mask multiply on its first image, but cores 1-7 get all-ones masks.

Layouts are precomputed on the host (numpy) so every DMA is a clean
contiguous 128-partition transfer.
"""

import ml_dtypes
import numpy as np

import concourse.bass as bass  # noqa: F401  (engine namespaces live on the nc object)
import concourse.tile as tile
from concourse import bacc, mybir
from concourse.bass_utils import run_bass_kernel_spmd

F32 = mybir.dt.float32
BF16 = mybir.dt.bfloat16
NP_BF16 = ml_dtypes.bfloat16
EPS = 1e-5
B, C, H, W = 4096, 64, 7, 7
NCORES = 8
BPC = B // NCORES          # 512 images per core
SLOTS = BPC // 2           # 256 image slots per half-stream
NBUF = 3                   # padded-tile ring depth
NWARM = 60                 # HAM warmup matmuls
# Quads: (base_slot, n_images_per_chain). A quad = 2 pairs = 4 matmul
# chains of n images each (4n images). n=10 gives N=490 matmuls (one PSUM
# bank holds 512 fp32), minimizing the number of rounds and so the
# per-round LDWEIGHTS tax; 512 = 12*40 + 32, so the last quad has n=8.
QUADS = [(20 * v, 10) for v in range(12)] + [(240, 8)]
NMAX = 10                  # max images per chain

# (pair_in_quad, half, colgroup): the 4 concurrent chains of a quad.
# Even pair writes PSUM naturally, odd pair swapped - this alternation is
# what keeps all four PE quadrants busy across consecutive chains. Order
# within a round: the two streams sharing an XBUS column-group adjacent.
CHAINS = [(0, 0, 0), (1, 1, 0), (0, 1, 1), (1, 0, 1)]

_CACHE = {}


def _psum_view(ps, j, n):
    """[128, n, 7, 7] view of pair j's bank of a [128, 2, 512] psum tile."""
    return ps[:, j, 0:n * H * W].rearrange(
        "p (i h w) -> p i h w", i=n, h=H, w=W)


def _dedup_ldweights(nc):
    """Remove InstLdweights whose matmul reuses the weights already loaded
    into the same PE quadrant (the PE retains its stationary operand, so a
    matmul with no preceding LDWEIGHTS streams against the previously
    loaded weights - verified on hardware). Only the warmup matmuls repeat
    weights here; real tap matmuls each load their own."""
    kept = removed = 0
    for f in nc.m.functions:
        for b in f.blocks:
            insts = list(b.instructions)
            last = {}
            dead = []
            i = 0
            while i < len(insts):
                ins = insts[i]
                if isinstance(ins, mybir.InstLdweights):
                    assert i + 1 < len(insts), "trailing LDWEIGHTS"
                    mm = insts[i + 1]
                    assert isinstance(mm, mybir.InstMatmult), (
                        f"LDWEIGHTS not followed by matmul: {type(mm).__name__}")
                    sig = str(mm.ins[1])
                    tp = tuple(mm.tile_position)
                    if last.get(tp) == sig:
                        si = ins.sync_info
                        if si is not None and (len(si.on_wait) or len(si.on_update)):
                            msi = mm.sync_info
                            ow = list(si.on_wait)
                            ou = list(si.on_update)
                            if msi is not None:
                                ow += list(msi.on_wait)
                                ou += list(msi.on_update)
                            mm.sync_info = mybir.SyncInfo(on_wait=ow, on_update=ou)
                        dead.append(ins)
                        removed += 1
                    else:
                        last[tp] = sig
                        kept += 1
                    i += 2
                    continue
                assert not isinstance(ins, mybir.InstMatmult), "matmul without LDWEIGHTS"
                i += 1
            for d in dead:
                b.instructions.remove(d)
    assert kept + removed == 936 + NWARM and removed >= 50, (kept, removed)


def _emit_conv(nc, src_pad, w_sb, ps, n):
    """One quad of one conv: 4 concurrent 9-tap chains (36 matmuls)."""
    for t in range(9):
        dh, dw = t // 3, t % 3
        for (j, half, cg) in CHAINS:
            rhs = src_pad[64 * half:64 * half + 64,
                          n * j:n * (j + 1), dh:dh + 7, dw:dw + 7]
            lhsT = w_sb[64 * half:64 * half + 64, t, :]
            out = ps[64 * cg:64 * cg + 64, j, 0:n * H * W]
            nc.tensor.matmul(out, lhsT, rhs, start=(t == 0), stop=(t == 8))


def _build():
    nc = bacc.Bacc("TRN2", target_bir_lowering=False, debug=False,
                   num_devices=NCORES)
    x_d = nc.dram_tensor("x", [128, SLOTS, H, W], F32, kind="ExternalInput")
    w1_d = nc.dram_tensor("w1", [128, 9, 64], BF16, kind="ExternalInput")
    w2_d = nc.dram_tensor("w2", [128, 9, 64], BF16, kind="ExternalInput")
    cst_d = nc.dram_tensor("cst", [128, 2], F32, kind="ExternalInput")
    msk_d = nc.dram_tensor("msk", [64, 2, H, W], F32, kind="ExternalInput")
    o_d = nc.dram_tensor("o", [128, SLOTS, H, W], BF16, kind="ExternalOutput")

    with tile.TileContext(nc) as tc:
        with (
            tc.tile_pool(name="singles", bufs=1) as singles,
            tc.tile_pool(name="xin", bufs=6) as xin_pool,
            tc.tile_pool(name="outp", bufs=3) as out_pool,
            tc.tile_pool(name="pads", bufs=1) as pad_pool,
            tc.tile_pool(name="ps1", bufs=2, space="PSUM") as ps1_pool,
            tc.tile_pool(name="ps2", bufs=2, space="PSUM") as ps2_pool,
        ):
            w1_sb = singles.tile([128, 9, 64], BF16, name="w1_sb")
            w2_sb = singles.tile([128, 9, 64], BF16, name="w2_sb")
            cst_sb = singles.tile([128, 2], F32, name="cst_sb")
            msk_sb = singles.tile([64, 2, H, W], F32, name="msk_sb")
            warm_sb = singles.tile([128, 1], F32, name="warm_sb")
            wrm_src = singles.tile([128, 392], F32, name="wrm_src")
            wrm_bf = singles.tile([128, 64], BF16, name="wrm_bf")

            # Persistent zero-padded 9x9 image tiles: the border is zeroed
            # once and never rewritten (compute only touches the interior).
            xpads, y1pads = [], []
            for i in range(NBUF):
                xp = pad_pool.tile([128, 2 * NMAX, 9, 9], BF16,
                                   name=f"xpad{i}", tag=f"xpad{i}")
                yp = pad_pool.tile([128, 2 * NMAX, 9, 9], BF16,
                                   name=f"y1pad{i}", tag=f"y1pad{i}")
                xpads.append(xp)
                y1pads.append(yp)

            def emit_conv2(state):
                v, base, n, yp, xin_q = state
                ps2 = ps2_pool.tile([128, 2, 512], F32, name="ps2t")
                _emit_conv(nc, yp, w2_sb, ps2, n)
                if v == 0:
                    # critic mask 2 on conv2 output of batch element 0
                    tgt = ps2[0:64, 0, 0:H * W].rearrange(
                        "p (h w) -> p h w", h=H, w=W)
                    nc.vector.tensor_mul(tgt, tgt, msk_sb[:, 1, :, :])
                # residual adds in exact fp32 (x never passes through bf16)
                # back-to-back on DVE, then the two relu(psum+shift2) finals
                # split DVE/ACT so they run concurrently
                views = [_psum_view(ps2, j, n) for j in range(2)]
                out_q = out_pool.tile([128, 2 * NMAX, H, W], BF16,
                                      name="out_q")
                for j in range(2):
                    nc.vector.tensor_add(
                        views[j], views[j], xin_q[:, n * j:n * (j + 1)])
                nc.vector.tensor_scalar(
                    out_q[:, 0:n], views[0],
                    cst_sb[:, 1:2], 0.0,
                    mybir.AluOpType.add, mybir.AluOpType.max)
                nc.scalar.activation(
                    out=out_q[:, n:2 * n], in_=views[1],
                    func=mybir.ActivationFunctionType.Relu,
                    bias=cst_sb[:, 1:2], scale=1.0)
                if v == len(QUADS) - 1:
                    # final quad: split the tail across two DMA rings so
                    # the serial descriptor issues overlap
                    nc.sync.dma_start(o_d[:, base:base + n], out_q[:, 0:n])
                    nc.gpsimd.dma_start(o_d[:, base + n:base + 2 * n],
                                        out_q[:, n:2 * n])
                else:
                    nc.sync.dma_start(o_d[:, base:base + 2 * n],
                                      out_q[:, 0:2 * n])

            def emit_in_dma(v, q):
                base, n = QUADS[v]
                xin_q = xin_pool.tile([128, 2 * NMAX, H, W], F32,
                                      name="xin_q")
                q.dma_start(xin_q[:, 0:2 * n], x_d[:, base:base + 2 * n])
                return xin_q

            pending = None
            xin_qs = {}
            for v, (base, n) in enumerate(QUADS):
                if v == 0:
                    # ---------------- prologue ----------------
                    # DVE: warmup data + first pad border, then a
                    # strided-dst CAST warmup into xpad2 (its interior is
                    # garbage until the lazy memset + real copy at quad 2)
                    nc.vector.memset(wrm_src[:], 0.0)
                    nc.vector.tensor_copy(wrm_bf[:], wrm_src[:, 0:64])
                    nc.vector.memset(xpads[0][:], 0.0)
                    nc.vector.tensor_copy(
                        xpads[2][:, 0:8, 1:8, 1:8],
                        wrm_src[:].rearrange("p (i h w) -> p i h w",
                                             i=8, h=7, w=7))
                    # ACT ring: w1 first - it gates the first LDWEIGHTS -
                    # then the activation-table prewarm
                    nc.scalar.dma_start(w1_sb[:], w1_d[:])
                    nc.scalar.memzero(warm_sb[:])
                    # first quads' x: 0-1 on the sync ring, 2 on ACT
                    xin_qs[0] = emit_in_dma(0, nc.sync)
                    xin_qs[1] = emit_in_dma(1, nc.sync)
                    # constants ride the SWDGE ring
                    nc.gpsimd.dma_start(cst_sb[:], cst_d[:])
                    nc.gpsimd.dma_start(w2_sb[:], w2_d[:])
                    nc.gpsimd.dma_start(msk_sb[:], msk_d[:])
                    xin_qs[2] = emit_in_dma(2, nc.scalar)
                    # HAM warmup: dummy matmuls bridge the DMA wait so the
                    # real stream starts at 2.4 GHz (their LDWEIGHTS are
                    # deduped post-schedule)
                    ps_w = ps1_pool.tile([128, 2, 512], F32, name="ps1t")
                    for i in range(NWARM):
                        nc.tensor.matmul(ps_w[0:64, i % 2, 0:64],
                                         wrm_bf[0:64, :], wrm_bf[0:64, :],
                                         start=True, stop=True,
                                         skip_group_check=True)
                    # y1pad borders off the critical path
                    for yp in y1pads:
                        nc.gpsimd.memset(yp[:], 0.0)
                elif v + 2 < len(QUADS):
                    xin_qs[v + 2] = emit_in_dma(v + 2, nc.sync)
                xin_q = xin_qs.pop(v)
                xp = xpads[v % NBUF]
                if 1 <= v < NBUF:
                    # lazy border memset for the ring slots not covered in
                    # the prologue (keeps the critical path short)
                    nc.vector.memset(xp[:], 0.0)
                nc.vector.tensor_copy(xp[:, 0:2 * n, 1:8, 1:8],
                                      xin_q[:, 0:2 * n])
                ps1 = ps1_pool.tile([128, 2, 512], F32, name="ps1t")
                _emit_conv(nc, xp, w1_sb, ps1, n)
                yp = y1pads[v % NBUF]
                for j in range(2):
                    nc.scalar.activation(
                        out=yp[:, n * j:n * (j + 1), 1:8, 1:8],
                        in_=_psum_view(ps1, j, n),
                        func=mybir.ActivationFunctionType.Relu,
                        bias=cst_sb[:, 0:1], scale=1.0)
                if v == 0:
                    # critic mask 1 on relu(bn1(conv1)) of batch elem 0
                    tgt = yp[0:64, 0, 1:8, 1:8]
                    nc.vector.tensor_mul(tgt, tgt, msk_sb[:, 0, :, :])
                if pending is not None:
                    emit_conv2(pending)
                pending = (v, base, n, yp, xin_q)
            emit_conv2(pending)

    _dedup_ldweights(nc)
    nc.compile()
    return nc


def _get_nc():
    if "nc" not in _CACHE:
        _CACHE["nc"] = _build()
    return _CACHE["nc"]


def _host_pack(x, w1, g1, b1, m1, v1, w2, g2, b2, m2, v2, mask1, mask2):
    x = np.ascontiguousarray(np.asarray(x, np.float32))
    scale1 = np.asarray(g1, np.float32) / np.sqrt(np.asarray(v1, np.float32) + EPS)
    shift1 = np.asarray(b1, np.float32) - np.asarray(m1, np.float32) * scale1
    scale2 = np.asarray(g2, np.float32) / np.sqrt(np.asarray(v2, np.float32) + EPS)
    shift2 = np.asarray(b2, np.float32) - np.asarray(m2, np.float32) * scale2

    def pack_w(w, scale):
        ws = np.asarray(w, np.float32) * scale[:, None, None, None]
        # [co, ci, kh, kw] -> [ci, tap, co], duplicated into both halves
        lhsT = ws.transpose(1, 2, 3, 0).reshape(64, 9, 64)
        return np.ascontiguousarray(np.tile(lhsT, (2, 1, 1)).astype(NP_BF16))

    wdev1, wdev2 = pack_w(w1, scale1), pack_w(w2, scale2)
    cst = np.tile(np.stack([shift1, shift2], 1), (2, 1))
    cst = np.ascontiguousarray(cst.astype(np.float32))

    # [core, pair, half, img, ch, h, w] -> [core, half*ch, slot, h, w]
    # ragged: 24 pairs of 10 images per half, then 2 pairs of 8
    def pack_part(xp, npair, n):
        xr = xp.reshape(NCORES, npair, 2, n, C, H, W)
        return xr.transpose(0, 2, 4, 1, 3, 5, 6).reshape(
            NCORES, 128, npair * n, H, W)

    x_cores = x.reshape(NCORES, BPC, C, H, W)
    xdev = np.ascontiguousarray(np.concatenate(
        [pack_part(x_cores[:, 0:480], 24, 10),
         pack_part(x_cores[:, 480:512], 2, 8)], axis=2))

    msk0 = np.ascontiguousarray(
        np.stack([np.asarray(mask1, np.float32),
                  np.asarray(mask2, np.float32)], 1))
    msk1s = np.ones_like(msk0)

    in_maps = []
    for c in range(NCORES):
        in_maps.append({
            "x": xdev[c],
            "w1": wdev1,
            "w2": wdev2,
            "cst": cst,
            "msk": msk0 if c == 0 else msk1s,
        })
    return in_maps


def _host_unpack(results):
    o = np.stack([np.asarray(results[c]["o"], np.float32)
                  for c in range(NCORES)])

    def unpack_part(op, npair, n):
        op = op.reshape(NCORES, 2, C, npair, n, H, W)
        return op.transpose(0, 3, 1, 4, 2, 5, 6).reshape(
            NCORES, npair * 2 * n, C, H, W)

    out = np.concatenate([unpack_part(o[:, :, 0:240], 24, 10),
                          unpack_part(o[:, :, 240:256], 2, 8)], axis=1)
    return np.ascontiguousarray(out.reshape(B, C, H, W))


def run(trace=False, **inputs):
    nc = _get_nc()
    in_maps = _host_pack(**inputs)
    res = run_bass_kernel_spmd(nc, in_maps, core_ids=list(range(NCORES)),
                               trace=trace)
    return _host_unpack(res.results), res


def kernel(**inputs) -> np.ndarray:
    out, _ = run(trace=False, **inputs)
    return out


# revision 17
# speedup vs baseline: 1.4133x; 1.0337x over previous
"""Trainium2 Bass kernel: ResNet BasicBlock (conv3x3-BN-ReLU-mask-conv3x3-mask-BN-residual-ReLU).

Problem shape: x[4096, 64, 7, 7], both convs 64->64 3x3 pad 1.

Strategy (pure data parallel, 8 cores, 512 images/core):
  * Channels live on SBUF partitions. Two 64-channel image streams are
    stacked into the 128 partitions ("half0" -> partitions 0-63,
    "half1" -> 64-127) so elementwise engines run at full width.
  * A 3x3 conv is 9 shifted 64x64 matmuls accumulated in PSUM. Images are
    zero-padded to 9x9 on-chip; each tap reads a strided window of the
    padded tile. Matmul operands are bf16 (fp32 matmuls lower to two PE
    passes - 4x the cost); accumulation stays fp32 in PSUM.
  * The 128x128 PE array is split into 4 64x64 quadrants via the matmul
    base partitions (rhs base -> row group, psum base -> column group).
    Four independent tap-chains (2 pairs x 2 halves) run concurrently, so
    the array is fully utilized despite C=64.
  * BN scales are folded into the conv weights on the host; BN shifts are
    per-partition bias/scalar operands of ScalarE activations. The
    identity residual is added in exact fp32 by VectorE directly into
    PSUM before the final relu.
  * The output travels back as bf16 (<=2^-9 relative rounding - well
    inside the error budget), halving outbound HBM traffic; the last
    quad's output DMAs split across the sync and gpsimd rings so the
    kernel tail's serial descriptor issues overlap.
  * Prologue: w1 rides first on the ACT HWDGE ring (it gates the first
    LDWEIGHTS); quads 0-1 arrive PRE-PADDED in bf16 and DMA straight into
    their pad tiles (DVE copies crawl under the inbound DMA burst, so no
    cast sits on the startup critical path); ~60 dummy matmuls bridge the
    DMA wait so the PE HAM clock gate is already at 2.4 GHz when the real
    matmul stream begins.
  * The critic masks only touch batch element 0: every core runs the same
    mask multiply on its first image, but cores 1-7 get all-ones masks.

Layouts are precomputed on the host (numpy) so every DMA is a clean
contiguous 128-partition transfer.
"""

import ml_dtypes
import numpy as np

import concourse.bass as bass  # noqa: F401  (engine namespaces live on the nc object)
import concourse.tile as tile
from concourse import bacc, mybir
from concourse.bass_utils import run_bass_kernel_spmd

F32 = mybir.dt.float32
BF16 = mybir.dt.bfloat16
NP_BF16 = ml_dtypes.bfloat16
EPS = 1e-5
B, C, H, W = 4096, 64, 7, 7
NCORES = 8
BPC = B // NCORES          # 512 images per core
SLOTS = BPC // 2           # 256 image slots per half-stream
NBUF = 3                   # padded-tile ring depth
NWARM = 60                 # HAM warmup matmuls
# Quads: (base_slot, n_images_per_chain). A quad = 2 pairs = 4 matmul
# chains of n images each (4n images). n=10 gives N=490 matmuls (one PSUM
# bank holds 512 fp32), minimizing the number of rounds and so the
# per-round LDWEIGHTS tax; 512 = 12*40 + 32, so the last quad has n=8.
QUADS = [(20 * v, 10) for v in range(12)] + [(240, 8)]
NMAX = 10                  # max images per chain

# (pair_in_quad, half, colgroup): the 4 concurrent chains of a quad.
# Even pair writes PSUM naturally, odd pair swapped - this alternation is
# what keeps all four PE quadrants busy across consecutive chains. Order
# within a round: the two streams sharing an XBUS column-group adjacent.
CHAINS = [(0, 0, 0), (1, 1, 0), (0, 1, 1), (1, 0, 1)]

_CACHE = {}


def _psum_view(ps, j, n):
    """[128, n, 7, 7] view of pair j's bank of a [128, 2, 512] psum tile."""
    return ps[:, j, 0:n * H * W].rearrange(
        "p (i h w) -> p i h w", i=n, h=H, w=W)


def _dedup_ldweights(nc):
    """Remove InstLdweights whose matmul reuses the weights already loaded
    into the same PE quadrant (the PE retains its stationary operand, so a
    matmul with no preceding LDWEIGHTS streams against the previously
    loaded weights - verified on hardware). Only the warmup matmuls repeat
    weights here; real tap matmuls each load their own."""
    kept = removed = 0
    for f in nc.m.functions:
        for b in f.blocks:
            insts = list(b.instructions)
            last = {}
            dead = []
            i = 0
            while i < len(insts):
                ins = insts[i]
                if isinstance(ins, mybir.InstLdweights):
                    assert i + 1 < len(insts), "trailing LDWEIGHTS"
                    mm = insts[i + 1]
                    assert isinstance(mm, mybir.InstMatmult), (
                        f"LDWEIGHTS not followed by matmul: {type(mm).__name__}")
                    sig = str(mm.ins[1])
                    tp = tuple(mm.tile_position)
                    if last.get(tp) == sig:
                        si = ins.sync_info
                        if si is not None and (len(si.on_wait) or len(si.on_update)):
                            msi = mm.sync_info
                            ow = list(si.on_wait)
                            ou = list(si.on_update)
                            if msi is not None:
                                ow += list(msi.on_wait)
                                ou += list(msi.on_update)
                            mm.sync_info = mybir.SyncInfo(on_wait=ow, on_update=ou)
                        dead.append(ins)
                        removed += 1
                    else:
                        last[tp] = sig
                        kept += 1
                    i += 2
                    continue
                assert not isinstance(ins, mybir.InstMatmult), "matmul without LDWEIGHTS"
                i += 1
            for d in dead:
                b.instructions.remove(d)
    assert kept + removed == 936 + NWARM and removed >= 50, (kept, removed)


def _emit_conv(nc, src_pad, w_sb, ps, n):
    """One quad of one conv: 4 concurrent 9-tap chains (36 matmuls)."""
    for t in range(9):
        dh, dw = t // 3, t % 3
        for (j, half, cg) in CHAINS:
            rhs = src_pad[64 * half:64 * half + 64,
                          n * j:n * (j + 1), dh:dh + 7, dw:dw + 7]
            lhsT = w_sb[64 * half:64 * half + 64, t, :]
            out = ps[64 * cg:64 * cg + 64, j, 0:n * H * W]
            nc.tensor.matmul(out, lhsT, rhs, start=(t == 0), stop=(t == 8))


def _build():
    nc = bacc.Bacc("TRN2", target_bir_lowering=False, debug=False,
                   num_devices=NCORES)
    x_d = nc.dram_tensor("x", [128, SLOTS, H, W], F32, kind="ExternalInput")
    w1_d = nc.dram_tensor("w1", [128, 9, 64], BF16, kind="ExternalInput")
    w2_d = nc.dram_tensor("w2", [128, 9, 64], BF16, kind="ExternalInput")
    cst_d = nc.dram_tensor("cst", [128, 2], F32, kind="ExternalInput")
    msk_d = nc.dram_tensor("msk", [64, 2, H, W], F32, kind="ExternalInput")
    xp01_d = nc.dram_tensor("xp01", [128, 40, 9, 9], BF16,
                            kind="ExternalInput")
    o_d = nc.dram_tensor("o", [128, SLOTS, H, W], BF16, kind="ExternalOutput")

    with tile.TileContext(nc) as tc:
        with (
            tc.tile_pool(name="singles", bufs=1) as singles,
            tc.tile_pool(name="xin", bufs=6) as xin_pool,
            tc.tile_pool(name="outp", bufs=3) as out_pool,
            tc.tile_pool(name="pads", bufs=1) as pad_pool,
            tc.tile_pool(name="ps1", bufs=2, space="PSUM") as ps1_pool,
            tc.tile_pool(name="ps2", bufs=2, space="PSUM") as ps2_pool,
        ):
            w1_sb = singles.tile([128, 9, 64], BF16, name="w1_sb")
            w2_sb = singles.tile([128, 9, 64], BF16, name="w2_sb")
            cst_sb = singles.tile([128, 2], F32, name="cst_sb")
            msk_sb = singles.tile([64, 2, H, W], F32, name="msk_sb")
            warm_sb = singles.tile([128, 1], F32, name="warm_sb")
            wrm_bf = singles.tile([128, 64], BF16, name="wrm_bf")

            # Persistent zero-padded 9x9 image tiles: the border is zeroed
            # once and never rewritten (compute only touches the interior).
            xpads, y1pads = [], []
            for i in range(NBUF):
                xp = pad_pool.tile([128, 2 * NMAX, 9, 9], BF16,
                                   name=f"xpad{i}", tag=f"xpad{i}")
                yp = pad_pool.tile([128, 2 * NMAX, 9, 9], BF16,
                                   name=f"y1pad{i}", tag=f"y1pad{i}")
                xpads.append(xp)
                y1pads.append(yp)

            def emit_conv2(state):
                v, base, n, yp, xin_q = state
                ps2 = ps2_pool.tile([128, 2, 512], F32, name="ps2t")
                _emit_conv(nc, yp, w2_sb, ps2, n)
                if v == 0:
                    # critic mask 2 on conv2 output of batch element 0
                    tgt = ps2[0:64, 0, 0:H * W].rearrange(
                        "p (h w) -> p h w", h=H, w=W)
                    nc.vector.tensor_mul(tgt, tgt, msk_sb[:, 1, :, :])
                # residual adds in exact fp32 (x never passes through bf16)
                # back-to-back on DVE, then the two relu(psum+shift2) finals
                # split DVE/ACT so they run concurrently
                views = [_psum_view(ps2, j, n) for j in range(2)]
                out_q = out_pool.tile([128, 2 * NMAX, H, W], BF16,
                                      name="out_q")
                for j in range(2):
                    nc.vector.tensor_add(
                        views[j], views[j], xin_q[:, n * j:n * (j + 1)])
                nc.vector.tensor_scalar(
                    out_q[:, 0:n], views[0],
                    cst_sb[:, 1:2], 0.0,
                    mybir.AluOpType.add, mybir.AluOpType.max)
                nc.scalar.activation(
                    out=out_q[:, n:2 * n], in_=views[1],
                    func=mybir.ActivationFunctionType.Relu,
                    bias=cst_sb[:, 1:2], scale=1.0)
                if v == len(QUADS) - 1:
                    # final quad: split the tail across two DMA rings so
                    # the serial descriptor issues overlap
                    nc.sync.dma_start(o_d[:, base:base + n], out_q[:, 0:n])
                    nc.gpsimd.dma_start(o_d[:, base + n:base + 2 * n],
                                        out_q[:, n:2 * n])
                else:
                    nc.sync.dma_start(o_d[:, base:base + 2 * n],
                                      out_q[:, 0:2 * n])

            def emit_in_dma(v, q):
                base, n = QUADS[v]
                xin_q = xin_pool.tile([128, 2 * NMAX, H, W], F32,
                                      name="xin_q")
                q.dma_start(xin_q[:, 0:2 * n], x_d[:, base:base + 2 * n])
                return xin_q

            pending = None
            xin_qs = {}
            for v, (base, n) in enumerate(QUADS):
                if v == 0:
                    # ---------------- prologue ----------------
                    # DVE copies crawl while the inbound DMA burst is in
                    # flight, so quads 0-1 arrive PRE-PADDED in bf16 and
                    # DMA straight into their pad tiles (borders included)
                    nc.vector.memset(wrm_bf[:], 0.0)
                    # ACT ring: w1 first - it gates the first LDWEIGHTS -
                    # then the activation-table prewarm
                    nc.scalar.dma_start(w1_sb[:], w1_d[:])
                    nc.scalar.memzero(warm_sb[:])
                    # pre-padded quads 0-1 + their residual x on sync
                    nc.sync.dma_start(xpads[0][:], xp01_d[:, 0:20])
                    nc.sync.dma_start(xpads[1][:], xp01_d[:, 20:40])
                    xin_qs[0] = emit_in_dma(0, nc.sync)
                    xin_qs[1] = emit_in_dma(1, nc.sync)
                    # constants ride the SWDGE ring
                    nc.gpsimd.dma_start(cst_sb[:], cst_d[:])
                    nc.gpsimd.dma_start(w2_sb[:], w2_d[:])
                    nc.gpsimd.dma_start(msk_sb[:], msk_d[:])
                    xin_qs[2] = emit_in_dma(2, nc.scalar)
                    # HAM warmup: dummy matmuls bridge the DMA wait so the
                    # real stream starts at 2.4 GHz (their LDWEIGHTS are
                    # deduped post-schedule)
                    ps_w = ps1_pool.tile([128, 2, 512], F32, name="ps1t")
                    for i in range(NWARM):
                        nc.tensor.matmul(ps_w[0:64, i % 2, 0:64],
                                         wrm_bf[0:64, :], wrm_bf[0:64, :],
                                         start=True, stop=True,
                                         skip_group_check=True)
                    # y1pad borders off the critical path
                    for yp in y1pads:
                        nc.gpsimd.memset(yp[:], 0.0)
                elif v + 2 < len(QUADS):
                    xin_qs[v + 2] = emit_in_dma(v + 2, nc.sync)
                xin_q = xin_qs.pop(v)
                xp = xpads[v % NBUF]
                if v == 2:
                    # lazy border memset for the one ring slot not loaded
                    # pre-padded (keeps the critical path short)
                    nc.vector.memset(xp[:], 0.0)
                if v >= 2:
                    nc.vector.tensor_copy(xp[:, 0:2 * n, 1:8, 1:8],
                                          xin_q[:, 0:2 * n])
                ps1 = ps1_pool.tile([128, 2, 512], F32, name="ps1t")
                _emit_conv(nc, xp, w1_sb, ps1, n)
                yp = y1pads[v % NBUF]
                for j in range(2):
                    nc.scalar.activation(
                        out=yp[:, n * j:n * (j + 1), 1:8, 1:8],
                        in_=_psum_view(ps1, j, n),
                        func=mybir.ActivationFunctionType.Relu,
                        bias=cst_sb[:, 0:1], scale=1.0)
                if v == 0:
                    # critic mask 1 on relu(bn1(conv1)) of batch elem 0
                    tgt = yp[0:64, 0, 1:8, 1:8]
                    nc.vector.tensor_mul(tgt, tgt, msk_sb[:, 0, :, :])
                if pending is not None:
                    emit_conv2(pending)
                pending = (v, base, n, yp, xin_q)
            emit_conv2(pending)

    _dedup_ldweights(nc)
    nc.compile()
    return nc


def _get_nc():
    if "nc" not in _CACHE:
        _CACHE["nc"] = _build()
    return _CACHE["nc"]


def _host_pack(x, w1, g1, b1, m1, v1, w2, g2, b2, m2, v2, mask1, mask2):
    x = np.ascontiguousarray(np.asarray(x, np.float32))
    scale1 = np.asarray(g1, np.float32) / np.sqrt(np.asarray(v1, np.float32) + EPS)
    shift1 = np.asarray(b1, np.float32) - np.asarray(m1, np.float32) * scale1
    scale2 = np.asarray(g2, np.float32) / np.sqrt(np.asarray(v2, np.float32) + EPS)
    shift2 = np.asarray(b2, np.float32) - np.asarray(m2, np.float32) * scale2

    def pack_w(w, scale):
        ws = np.asarray(w, np.float32) * scale[:, None, None, None]
        # [co, ci, kh, kw] -> [ci, tap, co], duplicated into both halves
        lhsT = ws.transpose(1, 2, 3, 0).reshape(64, 9, 64)
        return np.ascontiguousarray(np.tile(lhsT, (2, 1, 1)).astype(NP_BF16))

    wdev1, wdev2 = pack_w(w1, scale1), pack_w(w2, scale2)
    cst = np.tile(np.stack([shift1, shift2], 1), (2, 1))
    cst = np.ascontiguousarray(cst.astype(np.float32))

    # [core, pair, half, img, ch, h, w] -> [core, half*ch, slot, h, w]
    # ragged: 24 pairs of 10 images per half, then 2 pairs of 8
    def pack_part(xp, npair, n):
        xr = xp.reshape(NCORES, npair, 2, n, C, H, W)
        return xr.transpose(0, 2, 4, 1, 3, 5, 6).reshape(
            NCORES, 128, npair * n, H, W)

    x_cores = x.reshape(NCORES, BPC, C, H, W)
    xdev = np.ascontiguousarray(np.concatenate(
        [pack_part(x_cores[:, 0:480], 24, 10),
         pack_part(x_cores[:, 480:512], 2, 8)], axis=2))

    xp01 = np.zeros((NCORES, 128, 40, 9, 9), NP_BF16)
    xp01[:, :, :, 1:8, 1:8] = xdev[:, :, 0:40]
    xp01 = np.ascontiguousarray(xp01)

    msk0 = np.ascontiguousarray(
        np.stack([np.asarray(mask1, np.float32),
                  np.asarray(mask2, np.float32)], 1))
    msk1s = np.ones_like(msk0)

    in_maps = []
    for c in range(NCORES):
        in_maps.append({
            "x": xdev[c],
            "w1": wdev1,
            "w2": wdev2,
            "cst": cst,
            "msk": msk0 if c == 0 else msk1s,
            "xp01": xp01[c],
        })
    return in_maps


def _host_unpack(results):
    o = np.stack([np.asarray(results[c]["o"], np.float32)
                  for c in range(NCORES)])

    def unpack_part(op, npair, n):
        op = op.reshape(NCORES, 2, C, npair, n, H, W)
        return op.transpose(0, 3, 1, 4, 2, 5, 6).reshape(
            NCORES, npair * 2 * n, C, H, W)

    out = np.concatenate([unpack_part(o[:, :, 0:240], 24, 10),
                          unpack_part(o[:, :, 240:256], 2, 8)], axis=1)
    return np.ascontiguousarray(out.reshape(B, C, H, W))


def run(trace=False, **inputs):
    nc = _get_nc()
    in_maps = _host_pack(**inputs)
    res = run_bass_kernel_spmd(nc, in_maps, core_ids=list(range(NCORES)),
                               trace=trace)
    return _host_unpack(res.results), res


def kernel(**inputs) -> np.ndarray:
    out, _ = run(trace=False, **inputs)
    return out
